# revision 1
# baseline (speedup 1.0000x reference)
"""Trainium2 Bass kernel for nn_DecoderRNN (LSTM decoder + big vocab projection).

Strategy (8 NeuronCores, SPMD):
  - The LSTM recurrence (B=32, T=64, H=512) is replicated on every core:
    its per-step cost is dominated by streaming W_hh through the PE, which is
    batch-size independent, so sharding batch would not help. Replication
    keeps every core self-sufficient (no collectives).
  - The output projection (fc) is tensor-parallel over the vocab dim:
    core c computes logits[:, :, 1250*c : 1250*(c+1)] and writes its own
    [32, 64, 1250] output slab; the host concatenates slabs.
  - Embedding lookup runs on-device via indirect (gather) DMA.
  - The input projection Xp = xs @ W_ih.T + b is computed in bulk up front
    (it has no recurrent dependency) and bounced through a DRAM scratch
    buffer, prefetched per-step during the recurrence.

Layouts:
  - Gates are computed in batch-layout [B=32, 4H] PSUM via
    out = h.T_tile.T @ W_hh.T  (stationary = h.T tiles, moving = W_hh.T),
    using float32r (full fp32 storage, 1 cycle/row at N>=256).
  - Gate column order is permuted host-side to [i | f | o | g] so one
    sigmoid activation covers cols [0:1536) and one tanh covers [1536:2048).
  - h is transposed each step via PE transpose-mode into hsT (h.T history),
    which doubles as the stationary operand for both the recurrence and fc.

kernel(**inputs) takes FULL unsharded inputs, returns FULL [32, 64, 10000].
"""

import sys

sys.path.insert(0, "/opt/trn_rl_repo")

import numpy as np

N_CORES = 8
B, T = 32, 64
E, H, V = 512, 512, 10000
G4 = 4 * H            # 2048
TB = T * B            # 2048
VSL = V // N_CORES    # 1250 vocab rows per core
VPAD = 1280           # padded so fc N-chunks are 512/512/256 (all >=256)

_PROGRAM = None


def _build_program():
    import concourse.bass as bass
    import concourse.tile as tile
    from concourse import bacc, mybir
    from concourse.masks import make_identity
    from contextlib import ExitStack

    f32 = mybir.dt.float32
    bf16 = mybir.dt.bfloat16
    f8e4 = mybir.dt.float8e4
    i32 = mybir.dt.int32
    AF = mybir.ActivationFunctionType
    DR = mybir.MatmulPerfMode.DoubleRow
    from concourse.alu_op_type import AluOpType

    nc = bacc.Bacc(
        "TRN2",
        target_bir_lowering=False,
        debug=False,
        num_devices=N_CORES,
    )

    features = nc.dram_tensor("features", [B, E], f32, kind="ExternalInput").ap()
    idx = nc.dram_tensor("idx", [TB], i32, kind="ExternalInput").ap()
    embed = nc.dram_tensor("embed", [V, E], f32, kind="ExternalInput").ap()
    wihT = nc.dram_tensor("wihT", [E, G4], bf16, kind="ExternalInput").ap()
    whhT8 = nc.dram_tensor("whhT8", [H, G4], f8e4, kind="ExternalInput").ap()
    bih = nc.dram_tensor("bih", [G4], bf16, kind="ExternalInput").ap()
    bhh = nc.dram_tensor("bhh", [G4], bf16, kind="ExternalInput").ap()
    fcwT = nc.dram_tensor("fcwT", [H, VPAD], bf16, kind="ExternalInput").ap()
    fcb = nc.dram_tensor("fcb", [VPAD], bf16, kind="ExternalInput").ap()
    onesv = nc.dram_tensor("onesv", [128], bf16, kind="ExternalInput").ap()
    out = nc.dram_tensor("out", [B, T, VSL], f32, kind="ExternalOutput").ap()
    # Output viewed as [t, b, v]: a 128-row t-major tb tile = 4 t planes.
    out_r = out.rearrange("b t v -> t b v")

    with tile.TileContext(nc) as tc, ExitStack() as ctx:
        # ---------------- persistent state ----------------
        state = ctx.enter_context(tc.tile_pool(name="state", bufs=1))
        # h.T history: block t holds h(t).T (written at the end of step t).
        # Layout [p, k, 32*t + b] = h(t)[b, 128*k + p]
        hsT = state.tile([128, 4, 32 * T], bf16, tag="hsT")
        # fp8 copies for the recurrence matmul (DoubleRow): pair layout
        # [p, P, i, .] = row 128*(2P+i)+p.  hsT8 = 16*h.T, whhT8 = 64*W_hh.T,
        # so the gates PSUM carries 1024x the true value (descaled in the
        # activation with scale=1/1024; xp tiles are stored 1024x too).
        hsT8 = state.tile([128, 2, 2, 32 * T], f8e4, tag="hsT8")
        whhT8_sb = state.tile([128, 2, 2, G4], f8e4, tag="whhT8")
        xsT = state.tile([128, 4, TB], bf16, tag="xsT")   # [p, k, tb] = xs[tb, 128k+p]
        wihT_sb = state.tile([128, 4, G4], bf16, tag="wihT")
        fcwT_sb = state.tile([128, 4, VPAD], bf16, tag="fcwT")
        fcb_sb = state.tile([1, VPAD], bf16, tag="fcb")
        bias128 = state.tile([128, G4], bf16, tag="bias128")
        fcb128 = state.tile([128, VPAD], f32, tag="fcb128")
        c_sb = state.tile([B, H], f32, tag="c")
        ident = state.tile([128, 128], f32, tag="ident")
        ident_b = state.tile([128, 128], bf16, tag="ident_b")
        ones = state.tile([1, 128], bf16, tag="ones")
        eye4 = state.tile([128, 32], bf16, tag="eye4")   # I32 stacked 4x

        make_identity(nc, ident[:])
        make_identity(nc, ident_b[:])
        for q in range(4):
            nc.sync.dma_start(eye4[32 * q : 32 * (q + 1), :], ident_b[0:32, 0:32])
        nc.vector.memset(c_sb[:], 0.0)

        nc.sync.dma_start(fcb_sb[:], fcb[None, :])
        nc.sync.dma_start(ones[:], onesv[None, :])

        # Xp production machinery (used from prologue AND steady state).
        # Produced m-tiles [128, 2048] live in an SBUF ring; step t injects
        # rows [32*(t%4), +32) of tile t//4 into its gates.
        xp_psum = ctx.enter_context(tc.tile_pool(name="xp_ps", bufs=2, space="PSUM"))
        xp_ring = ctx.enter_context(tc.tile_pool(name="xp_ring", bufs=3))
        xp_tiles = {}

        def xp_chunk_mms(m, cch):
            """PE part of one Xp chunk: psum = xs_tile @ W_ih.T[:, chunk]."""
            sl = slice(512 * cch, 512 * (cch + 1))
            ps = xp_psum.tile([128, 512], f32, tag="xp")
            for k in range(4):
                nc.tensor.matmul(
                    ps[:],
                    lhsT=xsT[:, k, 128 * m : 128 * (m + 1)],
                    rhs=wihT_sb[:, k, sl],
                    start=(k == 0),
                    stop=(k == 3),
                )
            return ps

        def xp_chunk_add(m, cch, ps):
            # xp = 1024*(xs @ W_ih.T) + bias1024 (bias inputs pre-scaled on host)
            sl = slice(512 * cch, 512 * (cch + 1))
            nc.vector.scalar_tensor_tensor(
                xp_tiles[m][:, sl], ps[:], 1024.0, bias128[:, sl],
                op0=AluOpType.mult, op1=AluOpType.add,
            )

        def produce_xp(m):
            """Xp m-tile (tb rows 128m..128m+128) = xs_tile @ W_ih.T + bias."""
            xp_m_tile = xp_ring.tile([128, G4], bf16, tag="xp_m")
            xp_tiles[m] = xp_m_tile
            for cch in range(4):
                ps = xp_chunk_mms(m, cch)
                xp_chunk_add(m, cch, ps)

        # xs_b outlives the prologue (in-loop transposes read it)
        xs_pool = ctx.enter_context(tc.tile_pool(name="xs", bufs=1))
        xs_b = xs_pool.tile([128, 16, E], bf16, tag="xs_b")

        # ---------------- prologue: gather + casts + xs.T ----------------
        with ExitStack() as pro:
            small_pool = pro.enter_context(tc.tile_pool(name="small", bufs=1))
            tp_psum = pro.enter_context(tc.tile_pool(name="tp_ps", bufs=2, space="PSUM"))

            idx_sb = small_pool.tile([128, 16], i32, tag="idx")
            bias1_sb = small_pool.tile([1, G4], bf16, tag="bias1")
            bias2_sb = small_pool.tile([1, G4], bf16, tag="bias2")

            nc.sync.dma_start(idx_sb[:], idx.rearrange("(m p) -> p m", p=128))
            nc.sync.dma_start(bias1_sb[:], bih[None, :])
            nc.sync.dma_start(bias2_sb[:], bhh[None, :])
            # big weight loads go behind the small DMAs; wihT first (needed
            # by the first Xp tiles), fcwT last (first used at step 4)
            nc.sync.dma_start(wihT_sb[:], wihT.rearrange("(k p) g -> p k g", p=128))
            nc.sync.dma_start(
                whhT8_sb[:], whhT8.rearrange("(P i p) g -> p P i g", P=2, i=2)
            )
            nc.sync.dma_start(fcwT_sb[:], fcwT.rearrange("(k p) v -> p k v", p=128))
            # bias128 = broadcast(b_ih + b_hh) via rank-1 matmuls
            for cch in range(4):
                sl = slice(512 * cch, 512 * (cch + 1))
                bp = xp_psum.tile([128, 512], f32, tag="xp")
                nc.tensor.matmul(bp[:], lhsT=ones[0:1, :], rhs=bias1_sb[0:1, sl],
                                 start=True, stop=False)
                nc.tensor.matmul(bp[:], lhsT=ones[0:1, :], rhs=bias2_sb[0:1, sl],
                                 start=False, stop=True)
                nc.vector.tensor_copy(bias128[:, sl], bp[:])
            # fcb128 = broadcast(fc_b) via rank-1 matmuls
            for c0, csz in ((0, 512), (512, 512), (1024, 256)):
                bp = xp_psum.tile([128, 512], f32, tag="xp")
                nc.tensor.matmul(bp[:, 0:csz], lhsT=ones[0:1, :],
                                 rhs=fcb_sb[0:1, c0 : c0 + csz], start=True, stop=True)
                nc.vector.tensor_copy(fcb128[:, c0 : c0 + csz], bp[:, 0:csz])

            if True:

                # Embedding gather pipelined with the xs.T transposes:
                # gather m -> 4 PE transposes of tile m. Features overwrite
                # rows 0..31 right after gather 0 so tile 0 unblocks first.
                def gather(m):
                    nc.gpsimd.indirect_dma_start(
                        out=xs_b[:, m, :],
                        out_offset=None,
                        in_=embed[:, :],
                        in_offset=bass.IndirectOffsetOnAxis(
                            ap=idx_sb[:, m : m + 1], axis=0
                        ),
                    )

                def transpose_m(m, pool=None, tag="pt"):
                    for e in range(4):
                        pt = (pool or tp_psum).tile([128, 128], bf16, tag=tag)
                        nc.tensor.transpose(
                            pt[:], xs_b[:, m, 128 * e : 128 * (e + 1)], ident_b[:]
                        )
                        nc.vector.tensor_copy(xsT[:, e, 128 * m : 128 * (m + 1)], pt[:])
                tc_transpose_m = transpose_m

                gather(0)
                nc.gpsimd.dma_start(xs_b[0:32, 0, :], features[:, :])
                for m in range(1, 16):
                    gather(m)
                transpose_m(0)
                transpose_m(1)

            # first two Xp tiles up front; the rest interleave with the steps
            produce_xp(0)
            produce_xp(1)

        # ---------------- main recurrence + interleaved fc/Xp ----------------
        work = ctx.enter_context(tc.tile_pool(name="work", bufs=3))
        g_psum = ctx.enter_context(tc.tile_pool(name="g_ps", bufs=3, space="PSUM"))
        h_psum = ctx.enter_context(tc.tile_pool(name="h_ps", bufs=1, space="PSUM"))
        fc_psum = ctx.enter_context(tc.tile_pool(name="fc_ps", bufs=2, space="PSUM"))
        lg_pool = ctx.enter_context(tc.tile_pool(name="lg", bufs=2))

        FC_CHUNKS = ((0, 512), (512, 512), (1024, 256))
        lg_tiles = {}

        def fc_chunk_mms(m, j):
            """PE part of fc chunk j for tb tile m (fills PE bubbles)."""
            if j == 0:
                lg_new = lg_pool.tile([128, VPAD], f32, tag="lg")
                lg_tiles[m] = lg_new
            c0, csz = FC_CHUNKS[j]
            fps = fc_psum.tile([128, 512], f32, tag="fc")
            for k in range(4):
                nc.tensor.matmul(
                    fps[:, 0:csz],
                    lhsT=hsT[:, k, 128 * m : 128 * (m + 1)],
                    rhs=fcwT_sb[:, k, c0 : c0 + csz],
                    start=(k == 0),
                    stop=(k == 3),
                )
            return fps

        def fc_chunk_finish(m, j, fps):
            c0, csz = FC_CHUNKS[j]
            nc.vector.tensor_add(
                lg_tiles[m][:, c0 : c0 + csz], fps[:, 0:csz], fcb128[:, c0 : c0 + csz]
            )
            if j == 2:
                # DRAM side is [4 t, 32 b, 1250 v]; SBUF side [128, 1250]
                # pairs element-stream-wise (partition p = 32*t_local + b).
                nc.sync.dma_start(
                    out_r[4 * m : 4 * (m + 1), :, :], lg_tiles[m][:, 0:VSL]
                )

        # gate chunk order in SBUF columns (host permutes): 0=i 1=f 2=o 3=g
        # issue order: f first (σf feeds the slow gpsimd f*c mul), then g
        # (tanh early), then i (σi gates the ig product), o last.
        CHUNK_ORDER = (1, 3, 0, 2)

        for t in range(T):
            q = t % 4
            if 2 + t <= 15:
                tc_transpose_m(2 + t, pool=h_psum, tag="hp")
            xp_m = xp_tiles[t // 4]
            nl = work.tile([B, G4], bf16, tag="nl")
            # fc filler chunk: logits tile t//4-1, one N-chunk per step
            # (shifted to q=1..3 so the last chunks land on tail steps);
            # its matmuls are emitted mid-chunk-loop to absorb PE bubbles
            # without delaying the dependent gate groups.
            fc_pending = None
            m_fc = t // 4 - 1
            for ci, cch in enumerate(CHUNK_ORDER):
                if ci == 2 and m_fc >= 0 and q > 0:
                    fc_pending = fc_chunk_mms(m_fc, q - 1)
                sl = slice(512 * cch, 512 * (cch + 1))
                # per-chunk PSUM tile: chunks overlap (MMs of chunk c+1 run
                # while chunk c's activation drains its own tile)
                gps = g_psum.tile([B, 512], f32, tag="g")
                # inject Xp via stacked-identity matmul (row strip q of xp_m)
                nc.tensor.matmul(
                    gps[:],
                    lhsT=eye4[32 * q : 32 * (q + 1), :],
                    rhs=xp_m[32 * q : 32 * (q + 1), sl],
                    start=True,
                    stop=(t == 0),
                    tile_position=(32 * q, 0),
                )
                if t > 0:
                    # h @ W_hh.T in fp8 DoubleRow: pair P covers k-tiles
                    # 2P,2P+1; each matmul yields 256 gate cols (rhs free
                    # is (2,256)=512, out free 256).
                    for half in (0, 1):
                        hsl = slice(
                            512 * cch + 256 * half, 512 * cch + 256 * (half + 1)
                        )
                        for P in (0, 1):
                            nc.tensor.matmul(
                                gps[:, 256 * half : 256 * (half + 1)],
                                lhsT=hsT8[:, P, :, 32 * (t - 1) : 32 * t],
                                rhs=whhT8_sb[:, P, :, hsl],
                                start=False,
                                stop=(half == 1 and P == 1),
                                perf_mode=DR,
                                skip_group_check=True,
                            )
                if cch == 3:
                    nc.scalar.activation(nl[:, sl], gps[:], AF.Tanh, scale=1.0 / 1024.0)
                elif cch == 2:
                    # o-gate (last chunk): halves, so h=o*tanh(c) starts sooner
                    for ah in (0, 1):
                        nc.scalar.activation(
                            nl[:, 1024 + 256 * ah : 1024 + 256 * (ah + 1)],
                            gps[:, 256 * ah : 256 * (ah + 1)],
                            AF.Sigmoid,
                            scale=1.0 / 1024.0,
                        )
                else:
                    nc.scalar.activation(
                        nl[:, sl], gps[:], AF.Sigmoid, scale=1.0 / 1024.0
                    )

            # c = sigmoid(f)*c + sigmoid(i)*tanh(g);  h = sigmoid(o)*tanh(c)
            fmul = work.tile([B, H], f32, tag="fmul")
            nc.gpsimd.tensor_mul(fmul[:], nl[:, 512:1024], c_sb[:])
            ig = work.tile([B, H], bf16, tag="ig")
            nc.vector.tensor_mul(ig[:], nl[:, 0:512], nl[:, 1536:2048])
            tanhc = work.tile([B, H], bf16, tag="tanhc")
            h_t = work.tile([B, H], bf16, tag="h")

            # Fused per-half tail: finish c/h for half k, transpose that half
            # and emit its fp8 h.T immediately, so the next step's P=0
            # DoubleRow matmuls launch while half 1 is still in flight.
            hp = h_psum.tile([128, 128], bf16, tag="hp")
            for half in (0, 1):
                hs = slice(256 * half, 256 * (half + 1))
                nc.vector.tensor_add(c_sb[:, hs], fmul[:, hs], ig[:, hs])
                nc.scalar.activation(tanhc[:, hs], c_sb[:, hs], AF.Tanh)
                nc.vector.tensor_mul(
                    h_t[:, hs], nl[:, 1024 + 256 * half : 1024 + 256 * (half + 1)],
                    tanhc[:, hs],
                )
                for k in (2 * half, 2 * half + 1):
                    nc.tensor.transpose(
                        hp[:, 32 * k : 32 * (k + 1)],
                        h_t[0:32, 128 * k : 128 * (k + 1)],
                        ident_b[0:32, 0:32],
                    )
                # fp8 copy (16*h.T) first: it gates the next step's DoubleRow
                # matmuls; the bf16 hsT copy (fc input) can lag.  half
                # doubles as the pair index P (k = 2*P + i).
                nc.vector.tensor_scalar_mul(
                    hsT8[:, half, :, 32 * t : 32 * (t + 1)],
                    hp[:, 64 * half : 64 * (half + 1)].rearrange(
                        "p (k b) -> p k b", k=2
                    ),
                    16.0,
                )
                nc.vector.tensor_copy(
                    hsT[:, 2 * half : 2 * half + 2, 32 * t : 32 * (t + 1)],
                    hp[:, 64 * half : 64 * (half + 1)].rearrange(
                        "p (k b) -> p k b", k=2
                    ),
                )
            # Xp filler chunk AFTER the transposes: keeps the PE streaming
            # across the step boundary (while the hsT8 copy lands), so the
            # next step's DoubleRow group starts at a warm p-state.
            xp_m_next = t // 4 + 2
            xp_ps_pending = None
            if xp_m_next <= 15:
                if t % 4 == 0:
                    new_xp = xp_ring.tile([128, G4], bf16, tag="xp_m")
                    xp_tiles[xp_m_next] = new_xp
                xp_ps_pending = xp_chunk_mms(xp_m_next, t % 4)

            if fc_pending is not None:
                fc_chunk_finish(m_fc, q - 1, fc_pending)
            if xp_ps_pending is not None:
                xp_chunk_add(xp_m_next, t % 4, xp_ps_pending)


        for j in range(3):
            fps = fc_chunk_mms(15, j)
            fc_chunk_finish(15, j, fps)

    nc.compile()
    return nc


def _get_program():
    global _PROGRAM
    if _PROGRAM is None:
        _PROGRAM = _build_program()
    return _PROGRAM


# PyTorch LSTM gate order is [i, f, g, o]; we reorder rows to [i, f, o, g] so
# one device-side sigmoid covers a contiguous [0:1536) column range.
def _gate_perm():
    return np.concatenate(
        [np.arange(0, H), np.arange(H, 2 * H), np.arange(3 * H, 4 * H), np.arange(2 * H, 3 * H)]
    )


def _make_in_maps(features, captions, embed_table, W_ih, W_hh, b_ih, b_hh, fc_W, fc_b):
    import ml_dtypes

    bf16 = ml_dtypes.bfloat16
    f8e4 = ml_dtypes.float8_e4m3
    perm = _gate_perm()
    features = np.ascontiguousarray(np.asarray(features, dtype=np.float32))
    cap = np.asarray(captions).astype(np.int32)                      # [B, T]
    embed = np.ascontiguousarray(np.asarray(embed_table, dtype=np.float32))
    wihT = np.ascontiguousarray(np.asarray(W_ih, dtype=np.float32)[perm].T.astype(bf16))
    # recurrence weights in fp8 e4m3, pre-scaled x64 (gates PSUM = 1024x true)
    whhT8 = np.ascontiguousarray(
        np.clip(np.asarray(W_hh, dtype=np.float32)[perm].T * 64.0, -240, 240).astype(f8e4)
    )
    # biases pre-scaled x1024 to match the 1024x PSUM/xp scale
    bih = np.ascontiguousarray(
        (np.asarray(b_ih, dtype=np.float32)[perm] * 1024.0).astype(bf16)
    )
    bhh = np.ascontiguousarray(
        (np.asarray(b_hh, dtype=np.float32)[perm] * 1024.0).astype(bf16)
    )
    fc_W = np.asarray(fc_W, dtype=np.float32)
    fc_b = np.asarray(fc_b, dtype=np.float32)

    # gather indices, t-major: xs row t*32+b = embed[captions[b, t-1]] for t>=1
    idx = np.zeros(TB, dtype=np.int32)
    idx[B:] = cap[:, : T - 1].T.reshape(-1)

    in_maps = []
    for c in range(N_CORES):
        sl = slice(VSL * c, VSL * (c + 1))
        fcwT = np.zeros((H, VPAD), dtype=bf16)
        fcwT[:, :VSL] = fc_W[sl].T.astype(bf16)
        fcbp = np.zeros(VPAD, dtype=bf16)
        fcbp[:VSL] = fc_b[sl].astype(bf16)
        in_maps.append(
            dict(
                features=features,
                idx=idx,
                embed=embed,
                wihT=wihT,
                whhT8=whhT8,
                bih=bih,
                bhh=bhh,
                fcwT=np.ascontiguousarray(fcwT),
                fcb=fcbp,
                onesv=np.ones(128, dtype=bf16),
            )
        )
    return in_maps


def _install_ntff_hook():
    """Wire up NTFF profiling: bass_utils wants antenv.axon_hooks, which this
    container lacks; build it from trn_agent_boot's ctypes hook."""
    import sys as _sys
    import types

    if "antenv.axon_hooks" in _sys.modules:
        return
    if "/root/.axon_site" not in _sys.path:
        _sys.path.insert(0, "/root/.axon_site")
    from trn_agent_boot.trn_boot import _ntff_profile_via_ctypes

    hook = _ntff_profile_via_ctypes("/opt/axon/libaxon_pjrt.so")
    mod = types.ModuleType("antenv.axon_hooks")
    mod._hook = hook
    mod.set_axon_ntff_profile_hook = lambda h: setattr(mod, "_hook", h)
    mod.get_axon_ntff_profile_hook = lambda: mod._hook
    _sys.modules["antenv.axon_hooks"] = mod

    # avoid S3 uploads from the trace path in this zero-egress container
    import concourse.bass_utils as bu

    bu.upload_artifacts = lambda tmpdir: f"local:{tmpdir}"


def run(inputs, trace=False, trace_cores=None):
    """Run on hardware; returns (full_output [B,T,V] f32, BassKernelResults)."""
    from concourse.bass_utils import run_bass_kernel_spmd

    if trace:
        _install_ntff_hook()

    nc = _get_program()
    in_maps = _make_in_maps(
        inputs["features"],
        inputs["captions"],
        inputs["embed_table"],
        inputs["W_ih"],
        inputs["W_hh"],
        inputs["b_ih"],
        inputs["b_hh"],
        inputs["fc_W"],
        inputs["fc_b"],
    )
    kwargs = {}
    if trace:
        import os
        import shutil

        shutil.rmtree("/tmp/bass_trace", ignore_errors=True)
        os.makedirs("/tmp/bass_trace", exist_ok=True)
        kwargs.update(trace=True, trace_cores=trace_cores or [0], tmpdir="/tmp/bass_trace")
    res = run_bass_kernel_spmd(nc, in_maps, core_ids=list(range(N_CORES)), **kwargs)
    full = np.concatenate([r["out"] for r in res.results], axis=2)
    return full, res


def kernel(**inputs) -> np.ndarray:
    out, _ = run(inputs, trace=False)
    return out



# revision 9
# speedup vs baseline: 1.1956x; 1.1956x over previous
"""Trainium2 Bass kernel for nn_DecoderRNN (LSTM decoder + big vocab projection).

Strategy (8 NeuronCores, SPMD):
  - LSTM recurrence (B=32, T=64, H=512) replicated on every core (its cost is
    batch-independent); output projection fc tensor-parallel over vocab:
    core c writes logits[:, :, 1250c:1250(c+1)], host concatenates.
  - The gate pre-activations for step t accumulate in a [32, 512] PSUM bank
    per gate chunk from ONE fp8 DoubleRow group with K=1024+:
        bias (rank-1 DR)  +  xs(t) @ W_ih.T (2 DR mms)  +  h(t-1) @ W_hh.T
        (2 DR mms).
    The bias/xs matmuls don't depend on h(t-1), so they are emitted at the end
    of step t-1 and stream through the PE while step t-1's nonlinear tail runs.
    This replaces the old bulk-Xp production + SBUF ring + per-step
    re-injection (which streamed every Xp element through the PE twice).
  - Scaling: xsT8/hsT8 = 16x values, W weights = 64x, bias rhs = 32x with a
    16x stationary -> gate PSUM = 1024x true; activations descale by 1/1024.
  - Gate chunk order in SBUF columns is [f | i | g | o]: sigmoid(f) (which
    heads the c critical path via f*c) completes first.
  - All elementwise tail math on the Vector engine (gpsimd only does the
    prologue embedding gather); fc drains emit bf16 and the host upcasts.

PSUM budget (8 banks): 5 gate banks (4 live + staging for next step's bias/xs
groups) + 2 fc banks + 1 transpose bank.

kernel(**inputs) takes FULL unsharded inputs, returns FULL [32, 64, 10000] f32.
"""

import sys

sys.path.insert(0, "/opt/trn_rl_repo")

import numpy as np

N_CORES = 8
B, T = 32, 64
E, H, V = 512, 512, 10000
G4 = 4 * H            # 2048
TB = T * B            # 2048
VSL = V // N_CORES    # 1250 vocab rows per core
VPAD = 1280           # padded so fc N-chunks are 512/512/256 (all >=256)

_PROGRAM = None


def _build_program():
    import concourse.bass as bass
    import concourse.tile as tile
    from concourse import bacc, mybir
    from concourse.masks import make_identity
    from contextlib import ExitStack

    f32 = mybir.dt.float32
    bf16 = mybir.dt.bfloat16
    f8e4 = mybir.dt.float8e4
    i32 = mybir.dt.int32
    AF = mybir.ActivationFunctionType
    DR = mybir.MatmulPerfMode.DoubleRow

    nc = bacc.Bacc(
        "TRN2",
        target_bir_lowering=False,
        debug=False,
        num_devices=N_CORES,
    )

    features = nc.dram_tensor("features", [B, E], f32, kind="ExternalInput").ap()
    idx = nc.dram_tensor("idx", [TB], i32, kind="ExternalInput").ap()
    embed = nc.dram_tensor("embed", [V, E], f32, kind="ExternalInput").ap()
    wih8 = nc.dram_tensor("wih8", [E, G4], f8e4, kind="ExternalInput").ap()
    whh8 = nc.dram_tensor("whh8", [H, G4], f8e4, kind="ExternalInput").ap()
    biasg = nc.dram_tensor("biasg", [G4], bf16, kind="ExternalInput").ap()
    fcwT = nc.dram_tensor("fcwT", [H, VPAD], bf16, kind="ExternalInput").ap()
    fcb = nc.dram_tensor("fcb", [VPAD], bf16, kind="ExternalInput").ap()
    onesv = nc.dram_tensor("onesv", [128], bf16, kind="ExternalInput").ap()
    out = nc.dram_tensor("out", [B, T, VSL], bf16, kind="ExternalOutput").ap()
    # Output viewed as [t, b, v]: a 128-row t-major tb tile = 4 t planes.
    out_r = out.rearrange("b t v -> t b v")

    with tile.TileContext(nc) as tc, ExitStack() as ctx:
        # ---------------- persistent state ----------------
        state = ctx.enter_context(tc.tile_pool(name="state", bufs=1))
        # h.T history: block t holds h(t).T (written at the end of step t).
        # Layout [p, k, 32*t + b] = h(t)[b, 128*k + p]
        hsT = state.tile([128, 4, 32 * T], bf16, tag="hsT")
        # fp8 16x copies for the DoubleRow recurrence matmuls: pair layout
        # [p, P, i, .] = contraction row 128*(2P+i)+p.
        hsT8 = state.tile([128, 2, 2, 32 * T], f8e4, tag="hsT8")
        xsT8 = state.tile([128, 2, 2, TB], f8e4, tag="xsT8")
        whh8_sb = state.tile([128, 2, 2, G4], f8e4, tag="whh8")
        wih8_sb = state.tile([128, 2, 2, G4], f8e4, tag="wih8")
        bias_sb = state.tile([1, G4], bf16, tag="biasg")
        fcwT_sb = state.tile([128, 4, VPAD], bf16, tag="fcwT")
        fcb_sb = state.tile([1, VPAD], bf16, tag="fcb")
        fcb128 = state.tile([128, VPAD], f32, tag="fcb128")
        c_sb = state.tile([B, H], f32, tag="c")
        ident_b = state.tile([128, 128], bf16, tag="ident_b")
        ones = state.tile([1, 128], bf16, tag="ones")

        make_identity(nc, ident_b[:])
        nc.vector.memset(c_sb[:], 0.0)

        nc.sync.dma_start(fcb_sb[:], fcb[None, :])
        nc.sync.dma_start(ones[:], onesv[None, :])
        nc.sync.dma_start(bias_sb[:], biasg[None, :])

        # ---------------- PSUM pools ----------------
        g_psum = ctx.enter_context(tc.tile_pool(name="g_ps", bufs=5, space="PSUM"))
        fc_psum = ctx.enter_context(tc.tile_pool(name="fc_ps", bufs=2, space="PSUM"))
        h_psum = ctx.enter_context(tc.tile_pool(name="h_ps", bufs=1, space="PSUM"))

        gate_tiles = {}

        def emit_bias_xs(t):
            """Open step t's four chunk groups: bias + xs(t) @ W_ih.T.
            These don't depend on h(t-1): they stream during step t-1's tail."""
            for cch in range(4):
                sl = slice(512 * cch, 512 * (cch + 1))
                gt = g_psum.tile([B, 512], f32, tag="g")
                gate_tiles[(t, cch)] = gt
                nc.tensor.matmul(
                    gt[:], lhsT=ones[0:1, 0:32], rhs=bias_sb[0:1, sl],
                    start=True, stop=False, skip_group_check=True,
                )
                for P in (0, 1):
                    nc.tensor.matmul(
                        gt[:],
                        lhsT=xsT8[:, P, :, 32 * t : 32 * (t + 1)],
                        rhs=wih8_sb[:, P, :, sl],
                        start=False,
                        stop=(t == 0 and P == 1),
                        perf_mode=DR,
                        skip_group_check=True,
                    )

        # xs_b outlives the prologue (in-loop transposes read it)
        xs_pool = ctx.enter_context(tc.tile_pool(name="xs", bufs=1))
        xs_b = xs_pool.tile([128, 16, E], bf16, tag="xs_b")

        def transpose_m(m):
            """xs tile m -> xsT8 (fp8, 16x) via PE transpose + scaled copy."""
            for e in range(4):
                pt = h_psum.tile([128, 128], bf16, tag="hp")
                nc.tensor.transpose(
                    pt[:], xs_b[:, m, 128 * e : 128 * (e + 1)], ident_b[:]
                )
                nc.vector.tensor_scalar_mul(
                    xsT8[:, e // 2, e % 2, 128 * m : 128 * (m + 1)], pt[:], 16.0
                )

        # ---------------- prologue: gather + weight loads ----------------
        with ExitStack() as pro:
            small_pool = pro.enter_context(tc.tile_pool(name="small", bufs=1))
            idx_sb = small_pool.tile([128, 16], i32, tag="idx")

            nc.sync.dma_start(idx_sb[:], idx.rearrange("(m p) -> p m", p=128))
            # big weight loads; wih8 first (needed by step-0 groups)
            nc.sync.dma_start(
                wih8_sb[:], wih8.rearrange("(P i p) g -> p P i g", P=2, i=2)
            )
            nc.sync.dma_start(
                whh8_sb[:], whh8.rearrange("(P i p) g -> p P i g", P=2, i=2)
            )
            nc.sync.dma_start(fcwT_sb[:], fcwT.rearrange("(k p) v -> p k v", p=128))
            # fcb128 = broadcast(fc_b) via rank-1 matmuls
            for c0, csz in ((0, 512), (512, 512), (1024, 256)):
                bp = fc_psum.tile([128, 512], f32, tag="fc")
                nc.tensor.matmul(bp[:, 0:csz], lhsT=ones[0:1, :],
                                 rhs=fcb_sb[0:1, c0 : c0 + csz], start=True, stop=True)
                nc.vector.tensor_copy(fcb128[:, c0 : c0 + csz], bp[:, 0:csz])

            def gather(m):
                nc.gpsimd.indirect_dma_start(
                    out=xs_b[:, m, :],
                    out_offset=None,
                    in_=embed[:, :],
                    in_offset=bass.IndirectOffsetOnAxis(
                        ap=idx_sb[:, m : m + 1], axis=0
                    ),
                )

            gather(0)
            nc.gpsimd.dma_start(xs_b[0:32, 0, :], features[:, :])
            for m in range(1, 16):
                gather(m)
            transpose_m(0)
            transpose_m(1)

            emit_bias_xs(0)

        # ---------------- main recurrence + interleaved fc ----------------
        work = ctx.enter_context(tc.tile_pool(name="work", bufs=3))
        lg_pool = ctx.enter_context(tc.tile_pool(name="lg", bufs=2))

        FC_CHUNKS = ((0, 512), (512, 512), (1024, 256))
        lg_tiles = {}

        def fc_chunk_mms(m, j):
            """PE part of fc chunk j for tb tile m (fills PE bubbles)."""
            if j == 0:
                lg_new = lg_pool.tile([128, VPAD], bf16, tag="lg")
                lg_tiles[m] = lg_new
            c0, csz = FC_CHUNKS[j]
            fps = fc_psum.tile([128, 512], f32, tag="fc")
            for k in range(4):
                nc.tensor.matmul(
                    fps[:, 0:csz],
                    lhsT=hsT[:, k, 128 * m : 128 * (m + 1)],
                    rhs=fcwT_sb[:, k, c0 : c0 + csz],
                    start=(k == 0),
                    stop=(k == 3),
                )
            return fps

        def fc_chunk_finish(m, j, fps):
            c0, csz = FC_CHUNKS[j]
            nc.vector.tensor_add(
                lg_tiles[m][:, c0 : c0 + csz], fps[:, 0:csz], fcb128[:, c0 : c0 + csz]
            )
            if j == 2:
                # DRAM side is [4 t, 32 b, 1250 v]; SBUF side [128, 1250]
                # pairs element-stream-wise (partition p = 32*t_local + b).
                nc.sync.dma_start(
                    out_r[4 * m : 4 * (m + 1), :, :], lg_tiles[m][:, 0:VSL]
                )

        # gate chunk order in SBUF columns (host permutes): 0=f 1=i 2=g 3=o
        for t in range(T):
            q = t % 4
            m = t // 4
            nl = work.tile([B, G4], bf16, tag="nl")

            # ---- close the chunk groups with h(t-1) @ W_hh.T; act ASAP ----
            for cch in range(4):
                gt = gate_tiles[(t, cch)]
                if t > 0:
                    for P in (0, 1):
                        nc.tensor.matmul(
                            gt[:],
                            lhsT=hsT8[:, P, :, 32 * (t - 1) : 32 * t],
                            rhs=whh8_sb[:, P, :, 512 * cch : 512 * (cch + 1)],
                            start=False,
                            stop=(P == 1),
                            perf_mode=DR,
                            skip_group_check=True,
                        )
                sl = slice(512 * cch, 512 * (cch + 1))
                nc.scalar.activation(
                    nl[:, sl], gt[:], AF.Tanh if cch == 2 else AF.Sigmoid,
                    scale=1.0 / 1024.0,
                )

            # ---- PE fillers that don't gate the tail ----
            fc_pending = None
            if q < 3 and m >= 1:
                fc_pending = fc_chunk_mms(m - 1, q)

            # ---- c/h update (vector + scalar), halves pipelined ----
            # c = sigmoid(f)*c + sigmoid(i)*tanh(g);  h = sigmoid(o)*tanh(c)
            fmul = work.tile([B, H], f32, tag="fmul")
            ig = work.tile([B, H], bf16, tag="ig")
            tanhc = work.tile([B, H], bf16, tag="tanhc")
            h_t = work.tile([B, H], bf16, tag="h")
            for half in (0, 1):
                hs = slice(256 * half, 256 * (half + 1))
                nc.vector.tensor_mul(fmul[:, hs], nl[:, hs], c_sb[:, hs])
            nc.vector.tensor_mul(ig[:], nl[:, 512:1024], nl[:, 1024:1536])
            hp = h_psum.tile([128, 128], bf16, tag="hp")
            for half in (0, 1):
                hs = slice(256 * half, 256 * (half + 1))
                nc.vector.tensor_add(c_sb[:, hs], fmul[:, hs], ig[:, hs])
                nc.scalar.activation(tanhc[:, hs], c_sb[:, hs], AF.Tanh)
                nc.vector.tensor_mul(
                    h_t[:, hs], nl[:, 1536 + 256 * half : 1536 + 256 * (half + 1)],
                    tanhc[:, hs],
                )
                for k in (2 * half, 2 * half + 1):
                    nc.tensor.transpose(
                        hp[:, 32 * k : 32 * (k + 1)],
                        h_t[0:32, 128 * k : 128 * (k + 1)],
                        ident_b[0:32, 0:32],
                    )
                # fp8 copy (16*h.T) first: it gates the next step's DoubleRow
                # matmuls; the bf16 hsT copy (fc input) can lag.  half
                # doubles as the pair index P (k = 2*P + i).
                nc.vector.tensor_scalar_mul(
                    hsT8[:, half, :, 32 * t : 32 * (t + 1)],
                    hp[:, 64 * half : 64 * (half + 1)].rearrange(
                        "p (k b) -> p k b", k=2
                    ),
                    16.0,
                )
                nc.vector.tensor_copy(
                    hsT[:, 2 * half : 2 * half + 2, 32 * t : 32 * (t + 1)],
                    hp[:, 64 * half : 64 * (half + 1)].rearrange(
                        "p (k b) -> p k b", k=2
                    ),
                )

            # ---- non-critical work after the tail ----
            if fc_pending is not None:
                fc_chunk_finish(m - 1, q, fc_pending)
            if 2 + t <= 15:
                transpose_m(2 + t)
            # open next step's chunk groups (bias + xs): PE streams these
            # while the hsT8 copies land.
            if t + 1 < T:
                emit_bias_xs(t + 1)

        for j in range(3):
            fps = fc_chunk_mms(15, j)
            fc_chunk_finish(15, j, fps)

    nc.compile()
    return nc


def _get_program():
    global _PROGRAM
    if _PROGRAM is None:
        _PROGRAM = _build_program()
    return _PROGRAM


# PyTorch LSTM gate order is [i, f, g, o]; we reorder rows to [f, i, g, o] so
# the f-sigmoid (head of the c-chain) is the first chunk to complete.
def _gate_perm():
    return np.concatenate(
        [np.arange(H, 2 * H), np.arange(0, H), np.arange(2 * H, 3 * H), np.arange(3 * H, 4 * H)]
    )


def _make_in_maps(features, captions, embed_table, W_ih, W_hh, b_ih, b_hh, fc_W, fc_b):
    import ml_dtypes

    bf16 = ml_dtypes.bfloat16
    f8e4 = ml_dtypes.float8_e4m3
    perm = _gate_perm()
    features = np.ascontiguousarray(np.asarray(features, dtype=np.float32))
    cap = np.asarray(captions).astype(np.int32)                      # [B, T]
    embed = np.ascontiguousarray(np.asarray(embed_table, dtype=np.float32))
    # weights in fp8 e4m3, pre-scaled x64; xs/h are 16x -> PSUM = 1024x true
    wih8 = np.ascontiguousarray(
        np.clip(np.asarray(W_ih, dtype=np.float32)[perm].T * 64.0, -240, 240).astype(f8e4)
    )
    whh8 = np.ascontiguousarray(
        np.clip(np.asarray(W_hh, dtype=np.float32)[perm].T * 64.0, -240, 240).astype(f8e4)
    )
    # bias via bf16 rank-1 matmul, pre-scaled x1024 to match the PSUM scale
    bsum = (np.asarray(b_ih, dtype=np.float32) + np.asarray(b_hh, dtype=np.float32))[perm]
    biasg = np.ascontiguousarray((bsum * 1024.0).astype(bf16))
    fc_W = np.asarray(fc_W, dtype=np.float32)
    fc_b = np.asarray(fc_b, dtype=np.float32)

    # gather indices, t-major: xs row t*32+b = embed[captions[b, t-1]] for t>=1
    idx = np.zeros(TB, dtype=np.int32)
    idx[B:] = cap[:, : T - 1].T.reshape(-1)

    in_maps = []
    for c in range(N_CORES):
        sl = slice(VSL * c, VSL * (c + 1))
        fcwT = np.zeros((H, VPAD), dtype=bf16)
        fcwT[:, :VSL] = fc_W[sl].T.astype(bf16)
        fcbp = np.zeros(VPAD, dtype=bf16)
        fcbp[:VSL] = fc_b[sl].astype(bf16)
        in_maps.append(
            dict(
                features=features,
                idx=idx,
                embed=embed,
                wih8=wih8,
                whh8=whh8,
                biasg=biasg,
                fcwT=np.ascontiguousarray(fcwT),
                fcb=fcbp,
                onesv=np.ones(128, dtype=bf16),
            )
        )
    return in_maps


def _install_ntff_hook():
    """Wire up NTFF profiling: bass_utils wants antenv.axon_hooks, which this
    container lacks; build it from trn_agent_boot's ctypes hook."""
    import sys as _sys
    import types

    if "antenv.axon_hooks" in _sys.modules:
        return
    if "/root/.axon_site" not in _sys.path:
        _sys.path.insert(0, "/root/.axon_site")
    from trn_agent_boot.trn_boot import _ntff_profile_via_ctypes

    hook = _ntff_profile_via_ctypes("/opt/axon/libaxon_pjrt.so")
    mod = types.ModuleType("antenv.axon_hooks")
    mod._hook = hook
    mod.set_axon_ntff_profile_hook = lambda h: setattr(mod, "_hook", h)
    mod.get_axon_ntff_profile_hook = lambda: mod._hook
    _sys.modules["antenv.axon_hooks"] = mod

    # avoid S3 uploads from the trace path in this zero-egress container
    import concourse.bass_utils as bu

    bu.upload_artifacts = lambda tmpdir: f"local:{tmpdir}"


def run(inputs, trace=False, trace_cores=None):
    """Run on hardware; returns (full_output [B,T,V] f32, BassKernelResults)."""
    from concourse.bass_utils import run_bass_kernel_spmd

    if trace:
        _install_ntff_hook()

    nc = _get_program()
    in_maps = _make_in_maps(
        inputs["features"],
        inputs["captions"],
        inputs["embed_table"],
        inputs["W_ih"],
        inputs["W_hh"],
        inputs["b_ih"],
        inputs["b_hh"],
        inputs["fc_W"],
        inputs["fc_b"],
    )
    kwargs = {}
    if trace:
        import os
        import shutil

        shutil.rmtree("/tmp/bass_trace", ignore_errors=True)
        os.makedirs("/tmp/bass_trace", exist_ok=True)
        kwargs.update(trace=True, trace_cores=trace_cores or [0], tmpdir="/tmp/bass_trace")
    res = run_bass_kernel_spmd(nc, in_maps, core_ids=list(range(N_CORES)), **kwargs)
    full = np.concatenate(
        [np.asarray(r["out"]).astype(np.float32) for r in res.results], axis=2
    )
    return full, res


def kernel(**inputs) -> np.ndarray:
    out, _ = run(inputs, trace=False)
    return out
